# revision 25
# baseline (speedup 1.0000x reference)
"""Per-row cosine similarity: out[b, n] = <a[b,n,:], b[b,n,:]> / (||a[b,n,:]|| * ||b[b,n,:]||).

Inputs a, b: [32, 2048, 1024] f32. Output: [32, 2048] f32.

Strategy: batch-shard across 8 NeuronCores (4 batches = 8192 rows per core).
Each core streams its 64 MiB through SBUF in [128 rows, 4x1024] super-tiles
(2 MiB per DMA) on the single Sync HWDGE ring. One ring keeps the 16 SDMA
engines on one packet stream (two concurrent rings interleave packets and
drop per-packet efficiency ~25%). Per 128-row tile, three fused
elementwise+row-sum ops:
  - dot(a,b): DVE scalar_tensor_tensor (mult + add-reduce, one instruction)
  - sum(a^2): ACT activation(Square, accum_out=...)
  - sum(b^2): alternates DVE/ACT per tile to balance engine load
ACT gets its a-only work (sum a^2) queued ahead of its b-dependent work so a
late b transfer cannot head-of-line-block it. A small epilogue computes
dot/sqrt(max(sa,eps)*max(sb,eps)) with a Newton-refined sqrt, transposes
[128, 64] -> [64, 128] on TensorE, and writes the 32 KiB result with one
contiguous DMA.
"""

import numpy as np

import concourse.bass as bass
import concourse.bacc as bacc
import concourse.mybir as mybir
import concourse.tile as tile
from concourse.bass_utils import run_bass_kernel_spmd
from concourse.masks import make_identity

N_CORES = 8
B, N, D = 32, 2048, 1024
ROWS_PER_CORE = (B // N_CORES) * N  # 8192
P = 128
T_SUPER = 6  # row-tiles per super-tile (3 MiB per input DMA)
N_TILES = ROWS_PER_CORE // P  # 64
IO_BUFS = 3
EPS = 1e-12

_cache: dict = {}
last_results = None  # BassKernelResults of the most recent run (for test harness)


def _build() -> bass.Bass:
    if "nc" in _cache:
        return _cache["nc"]

    f32 = mybir.dt.float32
    mult = mybir.AluOpType.mult
    add = mybir.AluOpType.add

    nc = bacc.Bacc(trn_type="TRN2")
    a_d = nc.dram_tensor("a", [ROWS_PER_CORE, D], f32, kind="ExternalInput")
    b_d = nc.dram_tensor("b", [ROWS_PER_CORE, D], f32, kind="ExternalInput")
    o_d = nc.dram_tensor("o", [ROWS_PER_CORE], f32, kind="ExternalOutput")

    # [p, tile, d] views; chunks slice the tile axis (ragged last super OK).
    a_v = a_d.rearrange("(u p) d -> p u d", p=P)
    b_v = b_d.rearrange("(u p) d -> p u d", p=P)
    # Full supers, then a ragged 4/2/2 tail so the post-stream compute
    # quantum (gated on the final transfer) stays small.
    schedule = []
    t0 = 0
    while N_TILES - t0 > T_SUPER:
        schedule.append((t0, T_SUPER))
        t0 += T_SUPER
    while t0 < N_TILES:
        rem = N_TILES - t0
        nt = 4 if rem > 4 else (2 if rem > 2 else rem)
        schedule.append((t0, nt))
        t0 += nt
    assert sum(nt for _, nt in schedule) == N_TILES and schedule[-1][1] <= 2

    with (
        tile.TileContext(nc) as tc,
        tc.tile_pool(name="io", bufs=IO_BUFS) as io,
        tc.tile_pool(name="scr", bufs=2) as scr,
        tc.tile_pool(name="aux", bufs=1) as aux,
        tc.tile_pool(name="ps", bufs=1, space="PSUM") as ps_pool,
    ):
        # Per-row statistics, one column per 128-row tile.
        dot = aux.tile([P, N_TILES], f32)
        sa = aux.tile([P, N_TILES], f32)
        sbE = aux.tile([P, N_TILES // 2], f32)  # sum(b^2), even tiles (DVE)
        sbO = aux.tile([P, N_TILES // 2], f32)  # sum(b^2), odd tiles (ACT)

        # The fused reduce ops must write their full-size elementwise result
        # somewhere; rotating scratch tiles keep consecutive ops independent.
        # (InstTensorTensorReduce and stride-0 broadcast outputs both crash the
        # exec unit on this runtime, so: scalar_tensor_tensor + real scratch.)
        def dve_dot(in0, in1, acc):
            dve_scr = scr.tile([P, D], f32, tag="dve_scr")
            nc.vector.scalar_tensor_tensor(
                out=dve_scr,
                in0=in0,
                scalar=1.0,
                in1=in1,
                op0=mult,
                op1=mult,
                accum_out=acc,
            )

        def act_sumsq(in0, acc):
            act_scr = scr.tile([P, D], f32, tag="act_scr")
            nc.scalar.activation(
                out=act_scr,
                in_=in0,
                func=mybir.ActivationFunctionType.Square,
                accum_out=acc,
            )

        # Epilogue: out = dot / sqrt(max(sa, EPS) * max(sb, EPS)), per row.
        # Stats column t maps to (i, par) with t = 2i+par. Split into halves
        # over tiles [0,32) / [32,64) so half 0 (compute + its 16 KiB store)
        # runs under the stream and only half remains after the last tile.
        ident = aux.tile([P, P], f32)
        make_identity(nc, ident)
        dotv = dot.rearrange("p (i par) -> p par i", par=2)
        sav = sa.rearrange("p (i par) -> p par i", par=2)
        o_v = o_d.rearrange("(t p) -> t p", p=P)
        W = N_TILES // 4  # 16 stat columns per par per half
        HT = N_TILES // 2  # 32 tiles per half

        def epilogue_half(h):
            i0 = h * W
            outT = aux.tile([P, HT], f32, tag="outT")
            outTv = outT.rearrange("p (w par) -> p w par", par=2)
            mA = aux.tile([P, W], f32, tag="mA")
            mB = aux.tile([P, W], f32, tag="mB")
            d2 = aux.tile([P, W], f32, tag="d2")
            sq = aux.tile([P, W], f32, tag="sq")
            rc = aux.tile([P, W], f32, tag="rc")
            t1 = aux.tile([P, W], f32, tag="t1")
            for par, sbH in ((0, sbE), (1, sbO)):
                nc.vector.tensor_scalar_max(mA, sav[:, par, i0 : i0 + W], EPS)
                nc.vector.tensor_scalar_max(mB, sbH[:, i0 : i0 + W], EPS)
                nc.vector.tensor_mul(d2, mA, mB)
                # sqrt with one Newton step: s1 = 0.5*(s + d2/s); ACT sqrt
                # alone has a loose ULP budget.
                nc.scalar.sqrt(sq, d2)
                nc.vector.reciprocal(rc, sq)
                nc.vector.tensor_mul(t1, d2, rc)
                nc.vector.tensor_add(t1, t1, sq)
                nc.vector.tensor_scalar_mul(t1, t1, 0.5)
                nc.vector.reciprocal(rc, t1)
                nc.vector.tensor_mul(
                    outTv[:, :, par], dotv[:, par, i0 : i0 + W], rc
                )
            # outT[p, tau] = result for tile t = h*32+tau, i.e. row t*128+p.
            # Transpose on TensorE so the store is one contiguous DMA.
            ps_t = ps_pool.tile([HT, P], f32, tag="ps_t")
            nc.tensor.transpose(ps_t, outT, ident)
            outF = aux.tile([HT, P], f32, tag="outF")
            nc.scalar.copy(outF, ps_t)
            nc.sync.dma_start(out=o_v[h * HT : (h + 1) * HT], in_=outF)

        for t0, nt in schedule:
            a_sb = io.tile([P, T_SUPER, D], f32, tag="a_sb")
            b_sb = io.tile([P, T_SUPER, D], f32, tag="b_sb")
            nc.sync.dma_start(out=a_sb[:, :nt, :], in_=a_v[:, t0 : t0 + nt, :])
            nc.sync.dma_start(out=b_sb[:, :nt, :], in_=b_v[:, t0 : t0 + nt, :])
            # ACT first sees its a-only ops (sum a^2), then the b-dependent
            # ones; DVE ops all need b anyway.
            for j in range(nt):
                t = t0 + j
                act_sumsq(a_sb[:, j, :], sa[:, t : t + 1])
            for j in range(nt):
                t = t0 + j
                aj = a_sb[:, j, :]
                bj = b_sb[:, j, :]
                dve_dot(aj, bj, dot[:, t : t + 1])
                if t % 2 == 0:
                    dve_dot(bj, bj, sbE[:, t // 2 : t // 2 + 1])
                else:
                    act_sumsq(bj, sbO[:, t // 2 : t // 2 + 1])
            if t0 < N_TILES // 2 <= t0 + nt:
                epilogue_half(0)

        epilogue_half(1)

    nc.finalize()
    _cache["nc"] = nc
    return nc


def kernel(a: np.ndarray, b: np.ndarray, trace: bool = False, **run_kwargs) -> np.ndarray:
    global last_results
    nc = _build()
    a = np.ascontiguousarray(np.asarray(a, dtype=np.float32)).reshape(
        N_CORES, ROWS_PER_CORE, D
    )
    b = np.ascontiguousarray(np.asarray(b, dtype=np.float32)).reshape(
        N_CORES, ROWS_PER_CORE, D
    )
    in_maps = [{"a": a[k], "b": b[k]} for k in range(N_CORES)]
    res = run_bass_kernel_spmd(
        nc, in_maps, core_ids=list(range(N_CORES)), trace=trace, **run_kwargs
    )
    last_results = res
    out = np.stack([res.results[k]["o"] for k in range(N_CORES)])
    return out.reshape(B, N).astype(np.float32, copy=False)



# revision 26
# speedup vs baseline: 1.1622x; 1.1622x over previous
"""Per-row cosine similarity: out[b, n] = <a[b,n,:], b[b,n,:]> / (||a[b,n,:]|| * ||b[b,n,:]||).

Inputs a, b: [32, 2048, 1024] f32. Output: [32, 2048] f32.

Strategy: batch-shard across 8 NeuronCores (4 batches = 8192 rows per core).
Each core streams its 64 MiB through SBUF in [128 rows, 4x1024] super-tiles
(2 MiB per DMA) on the single Sync HWDGE ring. One ring keeps the 16 SDMA
engines on one packet stream (two concurrent rings interleave packets and
drop per-packet efficiency ~25%). Per 128-row tile, three fused
elementwise+row-sum ops:
  - dot(a,b): DVE scalar_tensor_tensor (mult + add-reduce, one instruction)
  - sum(a^2): ACT activation(Square, accum_out=...)
  - sum(b^2): alternates DVE/ACT per tile to balance engine load
ACT gets its a-only work (sum a^2) queued ahead of its b-dependent work so a
late b transfer cannot head-of-line-block it. A small epilogue computes
dot/sqrt(max(sa,eps)*max(sb,eps)) with a Newton-refined sqrt, transposes
[128, 64] -> [64, 128] on TensorE, and writes the 32 KiB result with one
contiguous DMA.
"""

import numpy as np

import concourse.bass as bass
import concourse.bacc as bacc
import concourse.mybir as mybir
import concourse.tile as tile
from concourse.bass_utils import run_bass_kernel_spmd
from concourse.masks import make_identity

N_CORES = 8
B, N, D = 32, 2048, 1024
ROWS_PER_CORE = (B // N_CORES) * N  # 8192
P = 128
T_SUPER = 6  # row-tiles per super-tile (3 MiB per input DMA)
N_TILES = ROWS_PER_CORE // P  # 64
IO_BUFS = 3
EPS = 1e-12

_cache: dict = {}
last_results = None  # BassKernelResults of the most recent run (for test harness)


def _build() -> bass.Bass:
    if "nc" in _cache:
        return _cache["nc"]

    f32 = mybir.dt.float32
    mult = mybir.AluOpType.mult
    add = mybir.AluOpType.add

    nc = bacc.Bacc(trn_type="TRN2")
    a_d = nc.dram_tensor("a", [ROWS_PER_CORE, D], f32, kind="ExternalInput")
    b_d = nc.dram_tensor("b", [ROWS_PER_CORE, D], f32, kind="ExternalInput")
    o_d = nc.dram_tensor("o", [ROWS_PER_CORE], f32, kind="ExternalOutput")

    # [p, tile, d] views; chunks slice the tile axis (ragged last super OK).
    a_v = a_d.rearrange("(u p) d -> p u d", p=P)
    b_v = b_d.rearrange("(u p) d -> p u d", p=P)
    # Full supers, then a ragged 4/2/2 tail so the post-stream compute
    # quantum (gated on the final transfer) stays small.
    schedule = []
    t0 = 0
    while N_TILES - t0 > T_SUPER:
        schedule.append((t0, T_SUPER))
        t0 += T_SUPER
    while t0 < N_TILES:
        rem = N_TILES - t0
        nt = 4 if rem > 4 else (2 if rem > 2 else rem)
        schedule.append((t0, nt))
        t0 += nt
    assert sum(nt for _, nt in schedule) == N_TILES and schedule[-1][1] <= 2

    with (
        tile.TileContext(nc) as tc,
        tc.tile_pool(name="io", bufs=IO_BUFS) as io,
        tc.tile_pool(name="scr", bufs=2) as scr,
        tc.tile_pool(name="aux", bufs=1) as aux,
        tc.tile_pool(name="ps", bufs=1, space="PSUM") as ps_pool,
    ):
        # Per-row statistics, one column per 128-row tile.
        dot = aux.tile([P, N_TILES], f32)
        sa = aux.tile([P, N_TILES], f32)
        sbE = aux.tile([P, N_TILES // 2], f32)  # sum(b^2), even tiles (DVE)
        sbO = aux.tile([P, N_TILES // 2], f32)  # sum(b^2), odd tiles (ACT)

        # The fused reduce ops must write their full-size elementwise result
        # somewhere; rotating scratch tiles keep consecutive ops independent.
        # (InstTensorTensorReduce and stride-0 broadcast outputs both crash the
        # exec unit on this runtime, so: scalar_tensor_tensor + real scratch.)
        def dve_dot(in0, in1, acc):
            dve_scr = scr.tile([P, D], f32, tag="dve_scr")
            nc.vector.scalar_tensor_tensor(
                out=dve_scr,
                in0=in0,
                scalar=1.0,
                in1=in1,
                op0=mult,
                op1=mult,
                accum_out=acc,
            )

        def act_sumsq(in0, acc):
            act_scr = scr.tile([P, D], f32, tag="act_scr")
            nc.scalar.activation(
                out=act_scr,
                in_=in0,
                func=mybir.ActivationFunctionType.Square,
                accum_out=acc,
            )

        # Epilogue: out = dot / sqrt(max(sa, EPS) * max(sb, EPS)), per row.
        # Stats column t maps to (i, par) with t = 2i+par. Split into halves
        # over tiles [0,32) / [32,64) so half 0 (compute + its 16 KiB store)
        # runs under the stream and only half remains after the last tile.
        ident = aux.tile([P, P], f32)
        make_identity(nc, ident)
        dotv = dot.rearrange("p (i par) -> p par i", par=2)
        sav = sa.rearrange("p (i par) -> p par i", par=2)
        o_v = o_d.rearrange("(t p) -> t p", p=P)
        W = N_TILES // 4  # 16 stat columns per par per half
        HT = N_TILES // 2  # 32 tiles per half

        def epilogue_half(h):
            i0 = h * W
            outT = aux.tile([P, HT], f32, tag="outT")
            outTv = outT.rearrange("p (w par) -> p w par", par=2)
            mA = aux.tile([P, W], f32, tag="mA")
            mB = aux.tile([P, W], f32, tag="mB")
            d2 = aux.tile([P, W], f32, tag="d2")
            sq = aux.tile([P, W], f32, tag="sq")
            rc = aux.tile([P, W], f32, tag="rc")
            t1 = aux.tile([P, W], f32, tag="t1")
            for par, sbH in ((0, sbE), (1, sbO)):
                nc.vector.tensor_scalar_max(mA, sav[:, par, i0 : i0 + W], EPS)
                nc.vector.tensor_scalar_max(mB, sbH[:, i0 : i0 + W], EPS)
                nc.vector.tensor_mul(d2, mA, mB)
                # sqrt with one Newton step: s1 = 0.5*(s + d2/s); ACT sqrt
                # alone has a loose ULP budget.
                nc.scalar.sqrt(sq, d2)
                nc.vector.reciprocal(rc, sq)
                nc.vector.tensor_mul(t1, d2, rc)
                nc.vector.tensor_add(t1, t1, sq)
                nc.vector.tensor_scalar_mul(t1, t1, 0.5)
                nc.vector.reciprocal(rc, t1)
                nc.vector.tensor_mul(
                    outTv[:, :, par], dotv[:, par, i0 : i0 + W], rc
                )
            # outT[p, tau] = result for tile t = h*32+tau, i.e. row t*128+p.
            # Transpose on TensorE so the store is one contiguous DMA.
            ps_t = ps_pool.tile([HT, P], f32, tag="ps_t")
            nc.tensor.transpose(ps_t, outT, ident)
            outF = aux.tile([HT, P], f32, tag="outF")
            nc.scalar.copy(outF, ps_t)
            nc.sync.dma_start(out=o_v[h * HT : (h + 1) * HT], in_=outF)

        for t0, nt in schedule:
            a_sb = io.tile([P, T_SUPER, D], f32, tag="a_sb")
            b_sb = io.tile([P, T_SUPER, D], f32, tag="b_sb")
            nc.sync.dma_start(out=a_sb[:, :nt, :], in_=a_v[:, t0 : t0 + nt, :])
            nc.sync.dma_start(out=b_sb[:, :nt, :], in_=b_v[:, t0 : t0 + nt, :])
            # ACT first sees its a-only ops (sum a^2), then the b-dependent
            # ones; DVE ops all need b anyway.
            for j in range(nt):
                t = t0 + j
                act_sumsq(a_sb[:, j, :], sa[:, t : t + 1])
            for j in range(nt):
                t = t0 + j
                aj = a_sb[:, j, :]
                bj = b_sb[:, j, :]
                dve_dot(aj, bj, dot[:, t : t + 1])
                if t % 2 == 0:
                    dve_dot(bj, bj, sbE[:, t // 2 : t // 2 + 1])
                else:
                    act_sumsq(bj, sbO[:, t // 2 : t // 2 + 1])
        # Both epilogue halves run after the stream: interleaving epilogue
        # work into the in-order engine queues mid-stream delays the DVE/ACT
        # progress counters that gate input-DMA buffer reuse and stalls the
        # ring (measured: a ~20us, 91 GB/s crater).
        epilogue_half(0)
        epilogue_half(1)

    nc.finalize()
    _cache["nc"] = nc
    return nc


def kernel(a: np.ndarray, b: np.ndarray, trace: bool = False, **run_kwargs) -> np.ndarray:
    global last_results
    nc = _build()
    a = np.ascontiguousarray(np.asarray(a, dtype=np.float32)).reshape(
        N_CORES, ROWS_PER_CORE, D
    )
    b = np.ascontiguousarray(np.asarray(b, dtype=np.float32)).reshape(
        N_CORES, ROWS_PER_CORE, D
    )
    in_maps = [{"a": a[k], "b": b[k]} for k in range(N_CORES)]
    res = run_bass_kernel_spmd(
        nc, in_maps, core_ids=list(range(N_CORES)), trace=trace, **run_kwargs
    )
    last_results = res
    out = np.stack([res.results[k]["o"] for k in range(N_CORES)])
    return out.reshape(B, N).astype(np.float32, copy=False)

